# revision 20
# baseline (speedup 1.0000x reference)
"""Trainium2 Bass kernel for LoRAModulatedHyperformerPairBlock.

Shards the A (row) axis of edge_vec across 8 NeuronCores. The tiny
node-level math (LoRA weight generation, node layer-norm, left/right
projections) runs on host; each core then computes, for its 64 rows:

    edge1[a,b,:]  = right_aug[b,:] @ M_aug[a]          (rank-17 matmul)
    e             = edge_vec + edge1                   (+outp bias folded in M_aug)
    xhat          = layernorm(e)  (gamma/beta folded into t1 weights/bias)
    h             = relu(xhat @ W1g + b1')
    out           = e + h @ W2 + b2  (b2 added via K=1 matmul into PSUM)

All matmuls run in bf16 with fp32 PSUM accumulation; residual/LN state
stays fp32.
"""

import os
import sys

import numpy as np

for _p in ("/opt/trn_rl_repo",):
    if _p not in sys.path and os.path.isdir(_p):
        sys.path.insert(0, _p)

import ml_dtypes  # noqa: E402

import concourse.bass as bass  # noqa: E402
import concourse.tile as tile  # noqa: E402
from concourse import mybir  # noqa: E402
from concourse.bass import ts  # noqa: E402
from concourse.bass_utils import run_bass_kernel_spmd  # noqa: E402

A, F, C, M, R, NT = 512, 128, 16, 256, 4, 4
DT = F * NT  # 512
EPS = 1e-6
NCORES = 8
ASH = A // NCORES  # 64 rows per core
JB = A // 128  # 4 column blocks of 128 tokens

BF16 = ml_dtypes.bfloat16

_cached = {}
LAST_RESULTS = None

# Enable walrus LDW dedup (skips redundant stationary reloads); disabled by
# default in concourse but our matmul stream benefits from it.
if os.environ.get("KERNEL_LDW_OPT", "0") == "1":
    import concourse.bass_utils as _bu

    if not getattr(_bu, "_ldw_patched", False):
        _orig_run_command = _bu.run_command

        def _patched_run_command(argv, **kwargs):
            argv = [
                "--enable-ldw-opt=true" if a == "--enable-ldw-opt=false" else a
                for a in argv
            ]
            return _orig_run_command(argv, **kwargs)

        _bu.run_command = _patched_run_command
        _bu._ldw_patched = True


def _split_waits(nc, max_waits=1):
    """The pinned walrus build accepts at most one sem-wait per
    instruction; move extra waits onto preceding same-engine NOPs."""
    for fn in nc.m.functions:
        for bb in fn.blocks:
            insts = bb.instructions
            i = 0
            while i < len(insts):
                inst = insts[i]
                si = inst.sync_info
                if si is not None and len(si.on_wait) > max_waits:
                    waits = list(si.on_wait)
                    extra, keep = waits[:-max_waits], waits[-max_waits:]
                    nops = []
                    for k in range(0, len(extra), max_waits):
                        chunk = extra[k : k + max_waits]
                        nops.append(
                            mybir.InstNoOp(
                                name=f"{inst.name}-ws{k}",
                                sync_info=mybir.SyncInfo(on_wait=chunk, on_update=[]),
                                engine=inst.engine,
                                bass_nofuse=True,
                            )
                        )
                    inst.sync_info = mybir.SyncInfo(
                        on_wait=keep, on_update=list(si.on_update)
                    )
                    for n_, nop in enumerate(nops):
                        insts.insert(i + n_, nop)
                    i += len(nops)
                i += 1


def _build_bass():
    f32 = mybir.dt.float32
    bf16 = mybir.dt.bfloat16
    AOP = mybir.AluOpType
    ACTF = mybir.ActivationFunctionType
    from concourse.masks import make_identity

    nc = bass.Bass()
    edge_in = nc.declare_dram_parameter("edge_in", [ASH, A, F], f32, isOutput=False)
    m_t = nc.declare_dram_parameter("m_t", [C + 1, ASH, F], bf16, isOutput=False)
    right_t = nc.declare_dram_parameter("right_t", [C + 1, A], bf16, isOutput=False)
    w1 = nc.declare_dram_parameter("w1", [F, DT], bf16, isOutput=False)
    w2 = nc.declare_dram_parameter("w2", [F, NT, F], bf16, isOutput=False)
    b1 = nc.declare_dram_parameter("b1", [F, NT], f32, isOutput=False)
    b2 = nc.declare_dram_parameter("b2", [F, 1], f32, isOutput=False)
    edge_out = nc.declare_dram_parameter("edge_out", [ASH, A, F], f32, isOutput=True)

    with tile.TileContext(nc) as tc:
        from contextlib import ExitStack

        with ExitStack() as ctx:
            const = ctx.enter_context(tc.tile_pool(name="const", bufs=1))
            io = ctx.enter_context(tc.tile_pool(name="io", bufs=3))
            work = ctx.enter_context(tc.tile_pool(name="work", bufs=2))
            stats = ctx.enter_context(tc.tile_pool(name="stats", bufs=2))
            ps1 = ctx.enter_context(tc.tile_pool(name="ps1", bufs=1, space="PSUM"))
            psx = ctx.enter_context(tc.tile_pool(name="psx", bufs=2, space="PSUM"))
            psh = ctx.enter_context(tc.tile_pool(name="psh", bufs=2, space="PSUM"))
            pso = ctx.enter_context(tc.tile_pool(name="pso", bufs=1, space="PSUM"))
            pst = ctx.enter_context(tc.tile_pool(name="pst", bufs=1, space="PSUM"))

            # --- constants ---
            right_sb = const.tile([C + 1, A], bf16)
            nc.sync.dma_start(out=right_sb, in_=right_t[:])
            m_sb = const.tile([C + 1, ASH, F], bf16)
            nc.sync.dma_start(out=m_sb, in_=m_t[:])
            w1_sb = const.tile([F, DT], bf16)
            nc.sync.dma_start(out=w1_sb, in_=w1[:])
            w2_sb = const.tile([F, NT, F], bf16)
            nc.sync.dma_start(out=w2_sb, in_=w2[:])
            b1_sb = const.tile([F, NT], f32)
            nc.sync.dma_start(out=b1_sb, in_=b1[:])
            b2_sb = const.tile([F, 1], f32)
            nc.sync.dma_start(out=b2_sb, in_=b2[:])
            eps_sb = const.tile([128, 1], f32)
            nc.vector.memset(eps_sb, EPS)
            ident_bf = const.tile([128, 128], bf16)
            make_identity(nc, ident_bf)

            for ap_ in range(ASH // 2):
                a0 = 2 * ap_
                # edge1 for the row pair: 4 matmuls (K=17, N=256)
                pe1 = ps1.tile([128, JB, 2, F], f32, tag="pe1")
                for j in range(JB):
                    nc.tensor.matmul(
                        pe1[:, j],
                        lhsT=right_sb[:, ts(j, 128)],
                        rhs=m_sb[:, a0 : a0 + 2, :],
                        start=True,
                        stop=True,
                    )
                et = io.tile([128, JB, 2, F], f32, tag="et")
                for i in range(2):
                    nc.sync.dma_start(
                        out=et[:, :, i, :],
                        in_=edge_in[a0 + i].rearrange("(j p) f -> p j f", p=128),
                    )
                # e = edge_vec + edge1  (fp32 residual, both rows at once)
                e_pair = work.tile([128, JB, 2, F], f32, tag="e")
                nc.vector.tensor_add(out=e_pair, in0=pe1, in1=et)
                # LN stats per token block, micro-chain batched per pair
                mv = stats.tile([128, JB, 2, 2], f32, tag="mv")
                for i in range(2):
                    for j in range(JB):
                        st = stats.tile([128, 6], f32, tag="st")
                        nc.vector.bn_stats(out=st, in_=e_pair[:, j, i, :])
                        nc.vector.bn_aggr(out=mv[:, j, i, :], in_=st)
                std = stats.tile([128, JB, 2], f32, tag="std")
                nc.scalar.activation(
                    out=std,
                    in_=mv[:, :, :, 1],
                    func=ACTF.Sqrt,
                    bias=eps_sb[:, 0:1],
                    scale=1.0,
                )
                rstd = stats.tile([128, JB, 2], f32, tag="rstd")
                nc.vector.reciprocal(out=rstd, in_=std)
                bias_t = stats.tile([128, JB, 2], f32, tag="bt")
                nc.vector.scalar_tensor_tensor(
                    out=bias_t,
                    in0=mv[:, :, :, 0],
                    scalar=-1.0,
                    in1=rstd,
                    op0=AOP.mult,
                    op1=AOP.mult,
                )

                for i in range(2):
                    e_row = e_pair[:, :, i, :]
                    # xhat = (e - mean)*rstd ; split across DVE and ScalarE
                    xh = work.tile([128, JB, F], bf16, tag="xh")
                    for j in range(2):
                        nc.vector.tensor_scalar(
                            out=xh[:, j, :],
                            in0=e_row[:, j, :],
                            scalar1=mv[:, j, i, 0:1],
                            scalar2=rstd[:, j, i : i + 1],
                            op0=AOP.subtract,
                            op1=AOP.mult,
                        )
                    for j in range(2, JB):
                        nc.scalar.activation(
                            out=xh[:, j, :],
                            in_=e_row[:, j, :],
                            func=ACTF.Identity,
                            bias=bias_t[:, j, i : i + 1],
                            scale=rstd[:, j, i : i + 1],
                        )
                    # transpose xhat via PE -> [f, (j tok)]
                    xt = work.tile([128, JB, 128], bf16, tag="xt")
                    xtp = psx.tile([128, JB, 128], bf16, tag="xtp")
                    for j in range(JB):
                        nc.tensor.transpose(xtp[:, j, :], xh[:, j, :], ident_bf)
                    nc.vector.tensor_copy(out=xt, in_=xtp)

                    # t1: h_T[dt_k] = W1g_k^T @ xhat_T  (N=512)
                    h_sb = work.tile([128, NT, A], bf16, tag="h")
                    for k in range(NT):
                        ph = psh.tile([128, A], f32, tag="ph")
                        nc.tensor.matmul(
                            ph,
                            lhsT=w1_sb[:, ts(k, 128)],
                            rhs=xt,
                            start=True,
                            stop=True,
                        )
                        nc.scalar.activation(
                            out=h_sb[:, k, :],
                            in_=ph,
                            func=ACTF.Relu,
                            bias=b1_sb[:, k : k + 1],
                            scale=1.0,
                        )
                    # t2 (f-major): poT[f, tok] = sum_k W2_k^T @ h_k
                    poT = pso.tile([128, A], f32, tag="poT")
                    for k in range(NT):
                        nc.tensor.matmul(
                            poT,
                            lhsT=w2_sb[:, k, :],
                            rhs=h_sb[:, k, :],
                            start=(k == 0),
                            stop=(k == NT - 1),
                        )
                    # evacuate as bf16 (+b2 via bias port), transpose on PE
                    po_bf = work.tile([128, JB, 128], bf16, tag="po")
                    nc.scalar.activation(
                        out=po_bf,
                        in_=poT,
                        func=ACTF.Identity,
                        bias=b2_sb[:, 0:1],
                        scale=1.0,
                    )
                    pot = pst.tile([128, JB, F], bf16, tag="pot")
                    for j in range(JB):
                        nc.tensor.transpose(pot[:, j, :], po_bf[:, j, :], ident_bf)
                    ot = io.tile([128, JB, F], f32, tag="ot")
                    nc.vector.tensor_add(out=ot, in0=pot, in1=e_row)
                    nc.sync.dma_start(
                        out=edge_out[a0 + i].rearrange("(j p) f -> p j f", p=128),
                        in_=ot,
                    )

    _split_waits(nc)
    return nc


def _layer_norm_np(x, g, b, eps=EPS):
    m = x.mean(axis=-1, keepdims=True)
    v = ((x - m) ** 2).mean(axis=-1, keepdims=True)
    return (x - m) / np.sqrt(v + eps) * g + b


def _weff(p, mod):
    W = np.asarray(p["W"], np.float32)
    din, dout = W.shape
    Am = (mod @ np.asarray(p["Wa"], np.float32)).reshape(din, R)
    Bm = (mod @ np.asarray(p["Wb"], np.float32)).reshape(R, dout)
    return W + Am @ Bm, np.asarray(p["b"], np.float32)


def kernel(node_vec, edge_vec, node_mask, edge_mask, modulated_params, params):
    global LAST_RESULTS
    node_vec = np.asarray(node_vec, np.float32)
    edge_vec = np.asarray(edge_vec, np.float32)
    node_mask = np.asarray(node_mask, np.float32)
    mod = np.asarray(modulated_params, np.float32)

    # ---- host-side prep (tiny node-level math + weight folding) ----
    ln1_g = np.asarray(params["ln1_g"], np.float32)
    ln1_b = np.asarray(params["ln1_b"], np.float32)
    ln2_g = np.asarray(params["ln2_g"], np.float32)
    ln2_b = np.asarray(params["ln2_b"], np.float32)

    Wl, bl = _weff(params["left"], mod)
    Wr, br = _weff(params["right"], mod)
    Wo, bo = _weff(params["outp"], mod)
    W1, b1 = _weff(params["t1"], mod)
    W2, b2 = _weff(params["t2"], mod)

    act = _layer_norm_np(node_vec, ln1_g, ln1_b)  # [A,F]
    left = (act @ Wl + bl) * node_mask[:, None]  # [A,C]
    right = (act @ Wr + br) * node_mask[:, None]  # [A,C]

    # M[a,d,f] = sum_c left[a,c] * Wo[c*C+d, f]; bias row carries outp bias
    W3 = Wo.reshape(C, C, F)
    Mmat = np.einsum("ac,cdf->adf", left, W3, optimize=True)  # [A,C,F]
    M_aug = np.concatenate(
        [Mmat, np.broadcast_to(bo, (A, 1, F))], axis=1
    )  # [A,C+1,F]
    right_aug = np.concatenate([right, np.ones((A, 1), np.float32)], axis=1)

    # fold ln2 gamma/beta into t1
    W1g = ln2_g[:, None] * W1  # [F,DT]
    b1p = b1 + ln2_b @ W1  # [DT]

    right_t = np.ascontiguousarray(right_aug.T).astype(BF16)  # [C+1, A]
    w1_bf = np.ascontiguousarray(W1g).astype(BF16)  # [F, DT]
    w2_bf = np.ascontiguousarray(
        W2.reshape(NT, F, F).transpose(1, 0, 2)
    ).astype(BF16)  # [F, NT, F] : w2_bf[p,k,f] = W2[k*F+p, f]
    b1_h = np.ascontiguousarray(b1p.reshape(NT, F).T).astype(np.float32)  # [F,NT]
    b2_h = np.ascontiguousarray(b2.reshape(F, 1)).astype(np.float32)

    if "nc" not in _cached:
        _cached["nc"] = _build_bass()
    nc = _cached["nc"]

    in_maps = []
    for i in range(NCORES):
        sl = slice(i * ASH, (i + 1) * ASH)
        m_shard = np.ascontiguousarray(M_aug[sl].transpose(1, 0, 2)).astype(BF16)
        in_maps.append(
            {
                "edge_in": np.ascontiguousarray(edge_vec[sl]),
                "m_t": m_shard,
                "right_t": right_t,
                "w1": w1_bf,
                "w2": w2_bf,
                "b1": b1_h,
                "b2": b2_h,
            }
        )

    trace = os.environ.get("KERNEL_TRACE", "0") == "1"
    kwargs = {}
    if trace:
        kwargs["trace"] = True
        tmpdir = os.environ.get("KERNEL_TRACE_DIR")
        if tmpdir:
            kwargs["tmpdir"] = tmpdir
    res = run_bass_kernel_spmd(nc, in_maps, core_ids=list(range(NCORES)), **kwargs)
    LAST_RESULTS = res

    out = np.concatenate([res.results[i]["edge_out"] for i in range(NCORES)], axis=0)
    return out.astype(np.float32)


# revision 21
# speedup vs baseline: 1.0424x; 1.0424x over previous
"""Trainium2 Bass kernel for LoRAModulatedHyperformerPairBlock.

Shards the A (row) axis of edge_vec across 8 NeuronCores. The tiny
node-level math (LoRA weight generation, node layer-norm, left/right
projections) runs on host; each core then computes, for its 64 rows:

    edge1[a,b,:]  = right_aug[b,:] @ M_aug[a]          (rank-17 matmul)
    e             = edge_vec + edge1                   (+outp bias folded in M_aug)
    xhat          = layernorm(e)  (gamma/beta folded into t1 weights/bias)
    h             = relu(xhat @ W1g + b1')
    out           = e + h @ W2 + b2  (b2 added via K=1 matmul into PSUM)

All matmuls run in bf16 with fp32 PSUM accumulation; residual/LN state
stays fp32.
"""

import os
import sys

import numpy as np

for _p in ("/opt/trn_rl_repo",):
    if _p not in sys.path and os.path.isdir(_p):
        sys.path.insert(0, _p)

import ml_dtypes  # noqa: E402

import concourse.bass as bass  # noqa: E402
import concourse.tile as tile  # noqa: E402
from concourse import mybir  # noqa: E402
from concourse.bass import ts  # noqa: E402
from concourse.bass_utils import run_bass_kernel_spmd  # noqa: E402

A, F, C, M, R, NT = 512, 128, 16, 256, 4, 4
DT = F * NT  # 512
EPS = 1e-6
NCORES = 8
ASH = A // NCORES  # 64 rows per core
JB = A // 128  # 4 column blocks of 128 tokens

BF16 = ml_dtypes.bfloat16

_cached = {}
LAST_RESULTS = None

# Enable walrus LDW dedup (skips redundant stationary reloads); disabled by
# default in concourse but our matmul stream benefits from it.
if os.environ.get("KERNEL_LDW_OPT", "0") == "1":
    import concourse.bass_utils as _bu

    if not getattr(_bu, "_ldw_patched", False):
        _orig_run_command = _bu.run_command

        def _patched_run_command(argv, **kwargs):
            argv = [
                "--enable-ldw-opt=true" if a == "--enable-ldw-opt=false" else a
                for a in argv
            ]
            return _orig_run_command(argv, **kwargs)

        _bu.run_command = _patched_run_command
        _bu._ldw_patched = True


def _split_waits(nc, max_waits=1):
    """The pinned walrus build accepts at most one sem-wait per
    instruction; move extra waits onto preceding same-engine NOPs."""
    for fn in nc.m.functions:
        for bb in fn.blocks:
            insts = bb.instructions
            i = 0
            while i < len(insts):
                inst = insts[i]
                si = inst.sync_info
                if si is not None and len(si.on_wait) > max_waits:
                    waits = list(si.on_wait)
                    extra, keep = waits[:-max_waits], waits[-max_waits:]
                    nops = []
                    for k in range(0, len(extra), max_waits):
                        chunk = extra[k : k + max_waits]
                        nops.append(
                            mybir.InstNoOp(
                                name=f"{inst.name}-ws{k}",
                                sync_info=mybir.SyncInfo(on_wait=chunk, on_update=[]),
                                engine=inst.engine,
                                bass_nofuse=True,
                            )
                        )
                    inst.sync_info = mybir.SyncInfo(
                        on_wait=keep, on_update=list(si.on_update)
                    )
                    for n_, nop in enumerate(nops):
                        insts.insert(i + n_, nop)
                    i += len(nops)
                i += 1


def _build_bass():
    f32 = mybir.dt.float32
    bf16 = mybir.dt.bfloat16
    AOP = mybir.AluOpType
    ACTF = mybir.ActivationFunctionType
    from concourse.masks import make_identity

    nc = bass.Bass()
    edge_in = nc.declare_dram_parameter("edge_in", [ASH, A, F], f32, isOutput=False)
    m_t = nc.declare_dram_parameter("m_t", [C + 1, ASH, F], bf16, isOutput=False)
    right_t = nc.declare_dram_parameter("right_t", [C + 1, A], bf16, isOutput=False)
    w1 = nc.declare_dram_parameter("w1", [F, DT], bf16, isOutput=False)
    w2 = nc.declare_dram_parameter("w2", [F, NT, F], bf16, isOutput=False)
    b1 = nc.declare_dram_parameter("b1", [F, NT], f32, isOutput=False)
    b2 = nc.declare_dram_parameter("b2", [F, 1], f32, isOutput=False)
    edge_out = nc.declare_dram_parameter("edge_out", [ASH, A, F], f32, isOutput=True)

    with tile.TileContext(nc) as tc:
        from contextlib import ExitStack

        with ExitStack() as ctx:
            const = ctx.enter_context(tc.tile_pool(name="const", bufs=1))
            io = ctx.enter_context(tc.tile_pool(name="io", bufs=3))
            work = ctx.enter_context(tc.tile_pool(name="work", bufs=2))
            stats = ctx.enter_context(tc.tile_pool(name="stats", bufs=2))
            ps1 = ctx.enter_context(tc.tile_pool(name="ps1", bufs=1, space="PSUM"))
            psx = ctx.enter_context(tc.tile_pool(name="psx", bufs=2, space="PSUM"))
            psh = ctx.enter_context(tc.tile_pool(name="psh", bufs=2, space="PSUM"))
            pso = ctx.enter_context(tc.tile_pool(name="pso", bufs=1, space="PSUM"))
            pst = ctx.enter_context(tc.tile_pool(name="pst", bufs=1, space="PSUM"))

            # --- constants ---
            right_sb = const.tile([C + 1, A], bf16)
            nc.sync.dma_start(out=right_sb, in_=right_t[:])
            m_sb = const.tile([C + 1, ASH, F], bf16)
            nc.sync.dma_start(out=m_sb, in_=m_t[:])
            w1_sb = const.tile([F, DT], bf16)
            nc.sync.dma_start(out=w1_sb, in_=w1[:])
            w2_sb = const.tile([F, NT, F], bf16)
            nc.sync.dma_start(out=w2_sb, in_=w2[:])
            b1_sb = const.tile([F, NT], f32)
            nc.sync.dma_start(out=b1_sb, in_=b1[:])
            b2_sb = const.tile([F, 1], f32)
            nc.sync.dma_start(out=b2_sb, in_=b2[:])
            eps_sb = const.tile([128, 1], f32)
            nc.vector.memset(eps_sb, EPS)
            ident_bf = const.tile([128, 128], bf16)
            make_identity(nc, ident_bf)

            for ap_ in range(ASH // 2):
                a0 = 2 * ap_
                # edge1 for the row pair: 4 matmuls (K=17, N=256)
                pe1 = ps1.tile([128, JB, 2, F], f32, tag="pe1")
                for j in range(JB):
                    nc.tensor.matmul(
                        pe1[:, j],
                        lhsT=right_sb[:, ts(j, 128)],
                        rhs=m_sb[:, a0 : a0 + 2, :],
                        start=True,
                        stop=True,
                    )
                et = io.tile([128, JB, 2, F], f32, tag="et")
                for i in range(2):
                    nc.sync.dma_start(
                        out=et[:, :, i, :],
                        in_=edge_in[a0 + i].rearrange("(j p) f -> p j f", p=128),
                    )
                # e = edge_vec + edge1  (fp32 residual, both rows at once)
                e_pair = work.tile([128, JB, 2, F], f32, tag="e")
                nc.vector.tensor_add(out=e_pair, in0=pe1, in1=et)
                # LN stats per token block, micro-chain batched per pair
                mv = stats.tile([128, JB, 2, 2], f32, tag="mv")
                for i in range(2):
                    for j in range(JB):
                        st = stats.tile([128, 6], f32, tag="st")
                        nc.vector.bn_stats(out=st, in_=e_pair[:, j, i, :])
                        nc.vector.bn_aggr(out=mv[:, j, i, :], in_=st)
                std = stats.tile([128, JB, 2], f32, tag="std")
                nc.scalar.activation(
                    out=std,
                    in_=mv[:, :, :, 1],
                    func=ACTF.Sqrt,
                    bias=eps_sb[:, 0:1],
                    scale=1.0,
                )
                rstd = stats.tile([128, JB, 2], f32, tag="rstd")
                nc.vector.reciprocal(out=rstd, in_=std)
                bias_t = stats.tile([128, JB, 2], f32, tag="bt")
                nc.vector.scalar_tensor_tensor(
                    out=bias_t,
                    in0=mv[:, :, :, 0],
                    scalar=-1.0,
                    in1=rstd,
                    op0=AOP.mult,
                    op1=AOP.mult,
                )

                for i in range(2):
                    e_row = e_pair[:, :, i, :]
                    # xhat = (e - mean)*rstd ; split across DVE and ScalarE
                    xh = work.tile([128, JB, F], bf16, tag="xh")
                    for j in range(2):
                        nc.vector.tensor_scalar(
                            out=xh[:, j, :],
                            in0=e_row[:, j, :],
                            scalar1=mv[:, j, i, 0:1],
                            scalar2=rstd[:, j, i : i + 1],
                            op0=AOP.subtract,
                            op1=AOP.mult,
                        )
                    for j in range(2, JB):
                        nc.scalar.activation(
                            out=xh[:, j, :],
                            in_=e_row[:, j, :],
                            func=ACTF.Identity,
                            bias=bias_t[:, j, i : i + 1],
                            scale=rstd[:, j, i : i + 1],
                        )
                    # transpose xhat via PE -> [f, (j tok)]
                    xt = work.tile([128, JB, 128], bf16, tag="xt")
                    xtp = psx.tile([128, JB, 128], bf16, tag="xtp")
                    for j in range(JB):
                        nc.tensor.transpose(xtp[:, j, :], xh[:, j, :], ident_bf)
                    nc.scalar.copy(out=xt, in_=xtp)

                    # t1: h_T[dt_k] = W1g_k^T @ xhat_T  (N=512)
                    h_sb = work.tile([128, NT, A], bf16, tag="h")
                    for k in range(NT):
                        ph = psh.tile([128, A], f32, tag="ph")
                        nc.tensor.matmul(
                            ph,
                            lhsT=w1_sb[:, ts(k, 128)],
                            rhs=xt,
                            start=True,
                            stop=True,
                        )
                        nc.scalar.activation(
                            out=h_sb[:, k, :],
                            in_=ph,
                            func=ACTF.Relu,
                            bias=b1_sb[:, k : k + 1],
                            scale=1.0,
                        )
                    # t2 (f-major): poT[f, tok] = sum_k W2_k^T @ h_k
                    poT = pso.tile([128, A], f32, tag="poT")
                    for k in range(NT):
                        nc.tensor.matmul(
                            poT,
                            lhsT=w2_sb[:, k, :],
                            rhs=h_sb[:, k, :],
                            start=(k == 0),
                            stop=(k == NT - 1),
                        )
                    # evacuate as bf16 (+b2 via bias port), transpose on PE
                    po_bf = work.tile([128, JB, 128], bf16, tag="po")
                    nc.scalar.activation(
                        out=po_bf,
                        in_=poT,
                        func=ACTF.Identity,
                        bias=b2_sb[:, 0:1],
                        scale=1.0,
                    )
                    pot = pst.tile([128, JB, F], bf16, tag="pot")
                    for j in range(JB):
                        nc.tensor.transpose(pot[:, j, :], po_bf[:, j, :], ident_bf)
                    ot = io.tile([128, JB, F], f32, tag="ot")
                    nc.vector.tensor_add(out=ot, in0=pot, in1=e_row)
                    nc.sync.dma_start(
                        out=edge_out[a0 + i].rearrange("(j p) f -> p j f", p=128),
                        in_=ot,
                    )

    _split_waits(nc)
    return nc


def _layer_norm_np(x, g, b, eps=EPS):
    m = x.mean(axis=-1, keepdims=True)
    v = ((x - m) ** 2).mean(axis=-1, keepdims=True)
    return (x - m) / np.sqrt(v + eps) * g + b


def _weff(p, mod):
    W = np.asarray(p["W"], np.float32)
    din, dout = W.shape
    Am = (mod @ np.asarray(p["Wa"], np.float32)).reshape(din, R)
    Bm = (mod @ np.asarray(p["Wb"], np.float32)).reshape(R, dout)
    return W + Am @ Bm, np.asarray(p["b"], np.float32)


def kernel(node_vec, edge_vec, node_mask, edge_mask, modulated_params, params):
    global LAST_RESULTS
    node_vec = np.asarray(node_vec, np.float32)
    edge_vec = np.asarray(edge_vec, np.float32)
    node_mask = np.asarray(node_mask, np.float32)
    mod = np.asarray(modulated_params, np.float32)

    # ---- host-side prep (tiny node-level math + weight folding) ----
    ln1_g = np.asarray(params["ln1_g"], np.float32)
    ln1_b = np.asarray(params["ln1_b"], np.float32)
    ln2_g = np.asarray(params["ln2_g"], np.float32)
    ln2_b = np.asarray(params["ln2_b"], np.float32)

    Wl, bl = _weff(params["left"], mod)
    Wr, br = _weff(params["right"], mod)
    Wo, bo = _weff(params["outp"], mod)
    W1, b1 = _weff(params["t1"], mod)
    W2, b2 = _weff(params["t2"], mod)

    act = _layer_norm_np(node_vec, ln1_g, ln1_b)  # [A,F]
    left = (act @ Wl + bl) * node_mask[:, None]  # [A,C]
    right = (act @ Wr + br) * node_mask[:, None]  # [A,C]

    # M[a,d,f] = sum_c left[a,c] * Wo[c*C+d, f]; bias row carries outp bias
    W3 = Wo.reshape(C, C, F)
    Mmat = np.einsum("ac,cdf->adf", left, W3, optimize=True)  # [A,C,F]
    M_aug = np.concatenate(
        [Mmat, np.broadcast_to(bo, (A, 1, F))], axis=1
    )  # [A,C+1,F]
    right_aug = np.concatenate([right, np.ones((A, 1), np.float32)], axis=1)

    # fold ln2 gamma/beta into t1
    W1g = ln2_g[:, None] * W1  # [F,DT]
    b1p = b1 + ln2_b @ W1  # [DT]

    right_t = np.ascontiguousarray(right_aug.T).astype(BF16)  # [C+1, A]
    w1_bf = np.ascontiguousarray(W1g).astype(BF16)  # [F, DT]
    w2_bf = np.ascontiguousarray(
        W2.reshape(NT, F, F).transpose(1, 0, 2)
    ).astype(BF16)  # [F, NT, F] : w2_bf[p,k,f] = W2[k*F+p, f]
    b1_h = np.ascontiguousarray(b1p.reshape(NT, F).T).astype(np.float32)  # [F,NT]
    b2_h = np.ascontiguousarray(b2.reshape(F, 1)).astype(np.float32)

    if "nc" not in _cached:
        _cached["nc"] = _build_bass()
    nc = _cached["nc"]

    in_maps = []
    for i in range(NCORES):
        sl = slice(i * ASH, (i + 1) * ASH)
        m_shard = np.ascontiguousarray(M_aug[sl].transpose(1, 0, 2)).astype(BF16)
        in_maps.append(
            {
                "edge_in": np.ascontiguousarray(edge_vec[sl]),
                "m_t": m_shard,
                "right_t": right_t,
                "w1": w1_bf,
                "w2": w2_bf,
                "b1": b1_h,
                "b2": b2_h,
            }
        )

    trace = os.environ.get("KERNEL_TRACE", "0") == "1"
    kwargs = {}
    if trace:
        kwargs["trace"] = True
        tmpdir = os.environ.get("KERNEL_TRACE_DIR")
        if tmpdir:
            kwargs["tmpdir"] = tmpdir
    res = run_bass_kernel_spmd(nc, in_maps, core_ids=list(range(NCORES)), **kwargs)
    LAST_RESULTS = res

    out = np.concatenate([res.results[i]["edge_out"] for i in range(NCORES)], axis=0)
    return out.astype(np.float32)


# revision 22
# speedup vs baseline: 1.0540x; 1.0111x over previous
"""Trainium2 Bass kernel for LoRAModulatedHyperformerPairBlock.

Shards the A (row) axis of edge_vec across 8 NeuronCores. The tiny
node-level math (LoRA weight generation, node layer-norm, left/right
projections) runs on host; each core then computes, for its 64 rows:

    edge1[a,b,:]  = right_aug[b,:] @ M_aug[a]          (rank-17 matmul)
    e             = edge_vec + edge1                   (+outp bias folded in M_aug)
    xhat          = layernorm(e)  (gamma/beta folded into t1 weights/bias)
    h             = relu(xhat @ W1g + b1')
    out           = e + h @ W2 + b2  (b2 added via K=1 matmul into PSUM)

All matmuls run in bf16 with fp32 PSUM accumulation; residual/LN state
stays fp32.
"""

import os
import sys

import numpy as np

for _p in ("/opt/trn_rl_repo",):
    if _p not in sys.path and os.path.isdir(_p):
        sys.path.insert(0, _p)

import ml_dtypes  # noqa: E402

import concourse.bass as bass  # noqa: E402
import concourse.tile as tile  # noqa: E402
from concourse import mybir  # noqa: E402
from concourse.bass import ts  # noqa: E402
from concourse.bass_utils import run_bass_kernel_spmd  # noqa: E402

A, F, C, M, R, NT = 512, 128, 16, 256, 4, 4
DT = F * NT  # 512
EPS = 1e-6
NCORES = 8
ASH = A // NCORES  # 64 rows per core
JB = A // 128  # 4 column blocks of 128 tokens

BF16 = ml_dtypes.bfloat16

_cached = {}
LAST_RESULTS = None

# Enable walrus LDW dedup (skips redundant stationary reloads); disabled by
# default in concourse but our matmul stream benefits from it.
if os.environ.get("KERNEL_LDW_OPT", "0") == "1":
    import concourse.bass_utils as _bu

    if not getattr(_bu, "_ldw_patched", False):
        _orig_run_command = _bu.run_command

        def _patched_run_command(argv, **kwargs):
            argv = [
                "--enable-ldw-opt=true" if a == "--enable-ldw-opt=false" else a
                for a in argv
            ]
            return _orig_run_command(argv, **kwargs)

        _bu.run_command = _patched_run_command
        _bu._ldw_patched = True


def _split_waits(nc, max_waits=1):
    """The pinned walrus build accepts at most one sem-wait per
    instruction; move extra waits onto preceding same-engine NOPs."""
    for fn in nc.m.functions:
        for bb in fn.blocks:
            insts = bb.instructions
            i = 0
            while i < len(insts):
                inst = insts[i]
                si = inst.sync_info
                if si is not None and len(si.on_wait) > max_waits:
                    waits = list(si.on_wait)
                    extra, keep = waits[:-max_waits], waits[-max_waits:]
                    nops = []
                    for k in range(0, len(extra), max_waits):
                        chunk = extra[k : k + max_waits]
                        nops.append(
                            mybir.InstNoOp(
                                name=f"{inst.name}-ws{k}",
                                sync_info=mybir.SyncInfo(on_wait=chunk, on_update=[]),
                                engine=inst.engine,
                                bass_nofuse=True,
                            )
                        )
                    inst.sync_info = mybir.SyncInfo(
                        on_wait=keep, on_update=list(si.on_update)
                    )
                    for n_, nop in enumerate(nops):
                        insts.insert(i + n_, nop)
                    i += len(nops)
                i += 1


def _build_bass():
    f32 = mybir.dt.float32
    bf16 = mybir.dt.bfloat16
    AOP = mybir.AluOpType
    ACTF = mybir.ActivationFunctionType
    from concourse.masks import make_identity

    nc = bass.Bass()
    edge_in = nc.declare_dram_parameter("edge_in", [ASH, A, F], f32, isOutput=False)
    m_t = nc.declare_dram_parameter("m_t", [C + 1, ASH, F], bf16, isOutput=False)
    right_t = nc.declare_dram_parameter("right_t", [C + 1, A], bf16, isOutput=False)
    w1 = nc.declare_dram_parameter("w1", [F, DT], bf16, isOutput=False)
    w2 = nc.declare_dram_parameter("w2", [F, NT, F], bf16, isOutput=False)
    b1 = nc.declare_dram_parameter("b1", [F, NT], f32, isOutput=False)
    b2 = nc.declare_dram_parameter("b2", [F, 1], f32, isOutput=False)
    edge_out = nc.declare_dram_parameter("edge_out", [ASH, A, F], f32, isOutput=True)

    with tile.TileContext(nc) as tc:
        from contextlib import ExitStack

        with ExitStack() as ctx:
            const = ctx.enter_context(tc.tile_pool(name="const", bufs=1))
            io = ctx.enter_context(tc.tile_pool(name="io", bufs=3))
            work = ctx.enter_context(tc.tile_pool(name="work", bufs=2))
            stats = ctx.enter_context(tc.tile_pool(name="stats", bufs=2))
            ps1 = ctx.enter_context(tc.tile_pool(name="ps1", bufs=1, space="PSUM"))
            psx = ctx.enter_context(tc.tile_pool(name="psx", bufs=2, space="PSUM"))
            psh = ctx.enter_context(tc.tile_pool(name="psh", bufs=2, space="PSUM"))
            pso = ctx.enter_context(tc.tile_pool(name="pso", bufs=1, space="PSUM"))
            pst = ctx.enter_context(tc.tile_pool(name="pst", bufs=1, space="PSUM"))

            # --- constants ---
            right_sb = const.tile([C + 1, A], bf16)
            nc.sync.dma_start(out=right_sb, in_=right_t[:])
            m_sb = const.tile([C + 1, ASH, F], bf16)
            nc.sync.dma_start(out=m_sb, in_=m_t[:])
            w1_sb = const.tile([F, DT], bf16)
            nc.sync.dma_start(out=w1_sb, in_=w1[:])
            w2_sb = const.tile([F, NT, F], bf16)
            nc.sync.dma_start(out=w2_sb, in_=w2[:])
            b1_sb = const.tile([F, NT], f32)
            nc.sync.dma_start(out=b1_sb, in_=b1[:])
            b2_sb = const.tile([F, 1], f32)
            nc.sync.dma_start(out=b2_sb, in_=b2[:])
            eps_sb = const.tile([128, 1], f32)
            nc.vector.memset(eps_sb, EPS)
            ident_bf = const.tile([128, 128], bf16)
            make_identity(nc, ident_bf)

            for ap_ in range(ASH // 2):
                a0 = 2 * ap_
                # edge1 for the row pair: 4 matmuls (K=17, N=256)
                pe1 = ps1.tile([128, 2, JB, F], f32, tag="pe1")
                for i in range(2):
                    for j in range(JB):
                        nc.tensor.matmul(
                            pe1[:, i, j, :],
                            lhsT=right_sb[:, ts(j, 128)],
                            rhs=m_sb[:, a0 + i, :],
                            start=True,
                            stop=True,
                        )
                et = io.tile([128, 2, JB, F], f32, tag="et")
                nc.sync.dma_start(
                    out=et,
                    in_=edge_in[a0 : a0 + 2].rearrange(
                        "a (j p) f -> p a j f", p=128
                    ),
                )
                # e = edge_vec + edge1  (fp32 residual, both rows at once)
                e_pair = work.tile([128, 2, JB, F], f32, tag="e")
                nc.vector.tensor_add(out=e_pair, in0=pe1, in1=et)
                # LN stats per token block, micro-chain batched per pair
                mv = stats.tile([128, 2, JB, 2], f32, tag="mv")
                for i in range(2):
                    for j in range(JB):
                        st = stats.tile([128, 6], f32, tag="st")
                        nc.vector.bn_stats(out=st, in_=e_pair[:, i, j, :])
                        nc.vector.bn_aggr(out=mv[:, i, j, :], in_=st)
                std = stats.tile([128, 2, JB], f32, tag="std")
                nc.scalar.activation(
                    out=std,
                    in_=mv[:, :, :, 1],
                    func=ACTF.Sqrt,
                    bias=eps_sb[:, 0:1],
                    scale=1.0,
                )
                rstd = stats.tile([128, 2, JB], f32, tag="rstd")
                nc.vector.reciprocal(out=rstd, in_=std)
                bias_t = stats.tile([128, 2, JB], f32, tag="bt")
                nc.vector.scalar_tensor_tensor(
                    out=bias_t,
                    in0=mv[:, :, :, 0],
                    scalar=-1.0,
                    in1=rstd,
                    op0=AOP.mult,
                    op1=AOP.mult,
                )

                for i in range(2):
                    e_row = e_pair[:, i, :, :]
                    # xhat = (e - mean)*rstd ; split across DVE and ScalarE
                    xh = work.tile([128, JB, F], bf16, tag="xh")
                    for j in range(2):
                        nc.vector.tensor_scalar(
                            out=xh[:, j, :],
                            in0=e_row[:, j, :],
                            scalar1=mv[:, i, j, 0:1],
                            scalar2=rstd[:, i, j : j + 1],
                            op0=AOP.subtract,
                            op1=AOP.mult,
                        )
                    for j in range(2, JB):
                        nc.scalar.activation(
                            out=xh[:, j, :],
                            in_=e_row[:, j, :],
                            func=ACTF.Identity,
                            bias=bias_t[:, i, j : j + 1],
                            scale=rstd[:, i, j : j + 1],
                        )
                    # transpose xhat via PE -> [f, (j tok)]
                    xt = work.tile([128, JB, 128], bf16, tag="xt")
                    xtp = psx.tile([128, JB, 128], bf16, tag="xtp")
                    for j in range(JB):
                        nc.tensor.transpose(xtp[:, j, :], xh[:, j, :], ident_bf)
                    nc.scalar.copy(out=xt, in_=xtp)

                    # t1: h_T[dt_k] = W1g_k^T @ xhat_T  (N=512)
                    h_sb = work.tile([128, NT, A], bf16, tag="h")
                    for k in range(NT):
                        ph = psh.tile([128, A], f32, tag="ph")
                        nc.tensor.matmul(
                            ph,
                            lhsT=w1_sb[:, ts(k, 128)],
                            rhs=xt,
                            start=True,
                            stop=True,
                        )
                        nc.scalar.activation(
                            out=h_sb[:, k, :],
                            in_=ph,
                            func=ACTF.Relu,
                            bias=b1_sb[:, k : k + 1],
                            scale=1.0,
                        )
                    # t2 (f-major): poT[f, tok] = sum_k W2_k^T @ h_k
                    poT = pso.tile([128, A], f32, tag="poT")
                    for k in range(NT):
                        nc.tensor.matmul(
                            poT,
                            lhsT=w2_sb[:, k, :],
                            rhs=h_sb[:, k, :],
                            start=(k == 0),
                            stop=(k == NT - 1),
                        )
                    # evacuate as bf16 (+b2 via bias port), transpose on PE
                    po_bf = work.tile([128, JB, 128], bf16, tag="po")
                    nc.scalar.activation(
                        out=po_bf,
                        in_=poT,
                        func=ACTF.Identity,
                        bias=b2_sb[:, 0:1],
                        scale=1.0,
                    )
                    pot = pst.tile([128, JB, F], bf16, tag="pot")
                    for j in range(JB):
                        nc.tensor.transpose(pot[:, j, :], po_bf[:, j, :], ident_bf)
                    ot = io.tile([128, JB, F], f32, tag="ot")
                    nc.vector.tensor_add(out=ot, in0=pot, in1=e_row)
                    nc.sync.dma_start(
                        out=edge_out[a0 + i].rearrange("(j p) f -> p j f", p=128),
                        in_=ot,
                    )

    _split_waits(nc)
    return nc


def _layer_norm_np(x, g, b, eps=EPS):
    m = x.mean(axis=-1, keepdims=True)
    v = ((x - m) ** 2).mean(axis=-1, keepdims=True)
    return (x - m) / np.sqrt(v + eps) * g + b


def _weff(p, mod):
    W = np.asarray(p["W"], np.float32)
    din, dout = W.shape
    Am = (mod @ np.asarray(p["Wa"], np.float32)).reshape(din, R)
    Bm = (mod @ np.asarray(p["Wb"], np.float32)).reshape(R, dout)
    return W + Am @ Bm, np.asarray(p["b"], np.float32)


def kernel(node_vec, edge_vec, node_mask, edge_mask, modulated_params, params):
    global LAST_RESULTS
    node_vec = np.asarray(node_vec, np.float32)
    edge_vec = np.asarray(edge_vec, np.float32)
    node_mask = np.asarray(node_mask, np.float32)
    mod = np.asarray(modulated_params, np.float32)

    # ---- host-side prep (tiny node-level math + weight folding) ----
    ln1_g = np.asarray(params["ln1_g"], np.float32)
    ln1_b = np.asarray(params["ln1_b"], np.float32)
    ln2_g = np.asarray(params["ln2_g"], np.float32)
    ln2_b = np.asarray(params["ln2_b"], np.float32)

    Wl, bl = _weff(params["left"], mod)
    Wr, br = _weff(params["right"], mod)
    Wo, bo = _weff(params["outp"], mod)
    W1, b1 = _weff(params["t1"], mod)
    W2, b2 = _weff(params["t2"], mod)

    act = _layer_norm_np(node_vec, ln1_g, ln1_b)  # [A,F]
    left = (act @ Wl + bl) * node_mask[:, None]  # [A,C]
    right = (act @ Wr + br) * node_mask[:, None]  # [A,C]

    # M[a,d,f] = sum_c left[a,c] * Wo[c*C+d, f]; bias row carries outp bias
    W3 = Wo.reshape(C, C, F)
    Mmat = np.einsum("ac,cdf->adf", left, W3, optimize=True)  # [A,C,F]
    M_aug = np.concatenate(
        [Mmat, np.broadcast_to(bo, (A, 1, F))], axis=1
    )  # [A,C+1,F]
    right_aug = np.concatenate([right, np.ones((A, 1), np.float32)], axis=1)

    # fold ln2 gamma/beta into t1
    W1g = ln2_g[:, None] * W1  # [F,DT]
    b1p = b1 + ln2_b @ W1  # [DT]

    right_t = np.ascontiguousarray(right_aug.T).astype(BF16)  # [C+1, A]
    w1_bf = np.ascontiguousarray(W1g).astype(BF16)  # [F, DT]
    w2_bf = np.ascontiguousarray(
        W2.reshape(NT, F, F).transpose(1, 0, 2)
    ).astype(BF16)  # [F, NT, F] : w2_bf[p,k,f] = W2[k*F+p, f]
    b1_h = np.ascontiguousarray(b1p.reshape(NT, F).T).astype(np.float32)  # [F,NT]
    b2_h = np.ascontiguousarray(b2.reshape(F, 1)).astype(np.float32)

    if "nc" not in _cached:
        _cached["nc"] = _build_bass()
    nc = _cached["nc"]

    in_maps = []
    for i in range(NCORES):
        sl = slice(i * ASH, (i + 1) * ASH)
        m_shard = np.ascontiguousarray(M_aug[sl].transpose(1, 0, 2)).astype(BF16)
        in_maps.append(
            {
                "edge_in": np.ascontiguousarray(edge_vec[sl]),
                "m_t": m_shard,
                "right_t": right_t,
                "w1": w1_bf,
                "w2": w2_bf,
                "b1": b1_h,
                "b2": b2_h,
            }
        )

    trace = os.environ.get("KERNEL_TRACE", "0") == "1"
    kwargs = {}
    if trace:
        kwargs["trace"] = True
        tmpdir = os.environ.get("KERNEL_TRACE_DIR")
        if tmpdir:
            kwargs["tmpdir"] = tmpdir
    res = run_bass_kernel_spmd(nc, in_maps, core_ids=list(range(NCORES)), **kwargs)
    LAST_RESULTS = res

    out = np.concatenate([res.results[i]["edge_out"] for i in range(NCORES)], axis=0)
    return out.astype(np.float32)


# revision 24
# speedup vs baseline: 1.0717x; 1.0168x over previous
"""Trainium2 Bass kernel for LoRAModulatedHyperformerPairBlock.

Shards the A (row) axis of edge_vec across 8 NeuronCores. The tiny
node-level math (LoRA weight generation, node layer-norm, left/right
projections) runs on host; each core then computes, for its 64 rows:

    edge1[a,b,:]  = right_aug[b,:] @ M_aug[a]          (rank-17 matmul)
    e             = edge_vec + edge1                   (+outp bias folded in M_aug)
    xhat          = layernorm(e)  (gamma/beta folded into t1 weights/bias)
    h             = relu(xhat @ W1g + b1')
    out           = e + h @ W2 + b2  (b2 via ScalarE bias port on PSUM evac)

All matmuls run in bf16 with fp32 PSUM accumulation; residual/LN state
stays fp32.
"""

import os
import sys

import numpy as np

for _p in ("/opt/trn_rl_repo",):
    if _p not in sys.path and os.path.isdir(_p):
        sys.path.insert(0, _p)

import ml_dtypes  # noqa: E402

import concourse.bass as bass  # noqa: E402
import concourse.tile as tile  # noqa: E402
from concourse import mybir  # noqa: E402
from concourse.bass import ts  # noqa: E402
from concourse.bass_utils import run_bass_kernel_spmd  # noqa: E402

A, F, C, M, R, NT = 512, 128, 16, 256, 4, 4
DT = F * NT  # 512
EPS = 1e-6
NCORES = 8
ASH = A // NCORES  # 64 rows per core
JB = A // 128  # 4 column blocks of 128 tokens

BF16 = ml_dtypes.bfloat16

_cached = {}
LAST_RESULTS = None

# Enable walrus LDW dedup (skips redundant stationary reloads); disabled by
# default in concourse but our matmul stream benefits from it.
if os.environ.get("KERNEL_LDW_OPT", "0") == "1":
    import concourse.bass_utils as _bu

    if not getattr(_bu, "_ldw_patched", False):
        _orig_run_command = _bu.run_command

        def _patched_run_command(argv, **kwargs):
            argv = [
                "--enable-ldw-opt=true" if a == "--enable-ldw-opt=false" else a
                for a in argv
            ]
            return _orig_run_command(argv, **kwargs)

        _bu.run_command = _patched_run_command
        _bu._ldw_patched = True


def _split_waits(nc, max_waits=1):
    """The pinned walrus build accepts at most one sem-wait per
    instruction; move extra waits onto preceding same-engine NOPs."""
    for fn in nc.m.functions:
        for bb in fn.blocks:
            insts = bb.instructions
            i = 0
            while i < len(insts):
                inst = insts[i]
                si = inst.sync_info
                if si is not None and len(si.on_wait) > max_waits:
                    waits = list(si.on_wait)
                    extra, keep = waits[:-max_waits], waits[-max_waits:]
                    nops = []
                    for k in range(0, len(extra), max_waits):
                        chunk = extra[k : k + max_waits]
                        nops.append(
                            mybir.InstNoOp(
                                name=f"{inst.name}-ws{k}",
                                sync_info=mybir.SyncInfo(on_wait=chunk, on_update=[]),
                                engine=inst.engine,
                                bass_nofuse=True,
                            )
                        )
                    inst.sync_info = mybir.SyncInfo(
                        on_wait=keep, on_update=list(si.on_update)
                    )
                    for n_, nop in enumerate(nops):
                        insts.insert(i + n_, nop)
                    i += len(nops)
                i += 1


def _build_bass():
    f32 = mybir.dt.float32
    bf16 = mybir.dt.bfloat16
    AOP = mybir.AluOpType
    ACTF = mybir.ActivationFunctionType
    from concourse.masks import make_identity

    nc = bass.Bass()
    edge_in = nc.declare_dram_parameter("edge_in", [ASH, A, F], f32, isOutput=False)
    m_t = nc.declare_dram_parameter("m_t", [C + 1, ASH, F], bf16, isOutput=False)
    right_t = nc.declare_dram_parameter("right_t", [C + 1, A], bf16, isOutput=False)
    w1 = nc.declare_dram_parameter("w1", [F, DT], bf16, isOutput=False)
    w2 = nc.declare_dram_parameter("w2", [F, NT, F], bf16, isOutput=False)
    b1 = nc.declare_dram_parameter("b1", [F, NT], f32, isOutput=False)
    b2 = nc.declare_dram_parameter("b2", [F, 1], f32, isOutput=False)
    edge_out = nc.declare_dram_parameter("edge_out", [ASH, A, F], f32, isOutput=True)

    with tile.TileContext(nc) as tc:
        from contextlib import ExitStack

        with ExitStack() as ctx:
            const = ctx.enter_context(tc.tile_pool(name="const", bufs=1))
            io = ctx.enter_context(tc.tile_pool(name="io", bufs=3))
            work = ctx.enter_context(tc.tile_pool(name="work", bufs=2))
            stats = ctx.enter_context(tc.tile_pool(name="stats", bufs=2))
            ps1 = ctx.enter_context(tc.tile_pool(name="ps1", bufs=1, space="PSUM"))
            psx = ctx.enter_context(tc.tile_pool(name="psx", bufs=2, space="PSUM"))
            psh = ctx.enter_context(tc.tile_pool(name="psh", bufs=2, space="PSUM"))
            pso = ctx.enter_context(tc.tile_pool(name="pso", bufs=1, space="PSUM"))
            pst = ctx.enter_context(tc.tile_pool(name="pst", bufs=1, space="PSUM"))

            # --- constants ---
            right_sb = const.tile([C + 1, A], bf16)
            nc.sync.dma_start(out=right_sb, in_=right_t[:])
            m_sb = const.tile([C + 1, ASH, F], bf16)
            nc.sync.dma_start(out=m_sb, in_=m_t[:])
            w1_sb = const.tile([F, DT], bf16)
            nc.sync.dma_start(out=w1_sb, in_=w1[:])
            w2_sb = const.tile([F, NT, F], bf16)
            nc.sync.dma_start(out=w2_sb, in_=w2[:])
            b1_sb = const.tile([F, NT], f32)
            nc.sync.dma_start(out=b1_sb, in_=b1[:])
            b2_sb = const.tile([F, 1], f32)
            nc.sync.dma_start(out=b2_sb, in_=b2[:])
            eps_sb = const.tile([128, 1], f32)
            nc.vector.memset(eps_sb, EPS)
            ident_bf = const.tile([128, 128], bf16)
            make_identity(nc, ident_bf)

            for ap_ in range(ASH // 2):
                a0 = 2 * ap_
                # edge1 for the row pair: 4 matmuls (K=17, N=256)
                pe1 = ps1.tile([128, JB, 2, F], f32, tag="pe1")
                for j in range(JB):
                    nc.tensor.matmul(
                        pe1[:, j],
                        lhsT=right_sb[:, ts(j, 128)],
                        rhs=m_sb[:, a0 : a0 + 2, :],
                        start=True,
                        stop=True,
                    )
                et = io.tile([128, 2, JB, F], f32, tag="et")
                nc.sync.dma_start(
                    out=et,
                    in_=edge_in[a0 : a0 + 2].rearrange(
                        "a (j p) f -> p a j f", p=128
                    ),
                )
                # e = edge_vec + edge1  (fp32 residual, both rows at once)
                e_pair = work.tile([128, 2, JB, F], f32, tag="e")
                nc.vector.tensor_add(
                    out=e_pair,
                    in0=pe1[:].rearrange("p j a f -> p a j f"),
                    in1=et,
                )
                # LN stats per token block, micro-chain batched per pair
                mv = stats.tile([128, 2, JB, 2], f32, tag="mv")
                for i in range(2):
                    for j in range(JB):
                        st = stats.tile([128, 6], f32, tag="st")
                        nc.vector.bn_stats(out=st, in_=e_pair[:, i, j, :])
                        nc.vector.bn_aggr(out=mv[:, i, j, :], in_=st)
                std = stats.tile([128, 2, JB], f32, tag="std")
                nc.scalar.activation(
                    out=std,
                    in_=mv[:, :, :, 1],
                    func=ACTF.Sqrt,
                    bias=eps_sb[:, 0:1],
                    scale=1.0,
                )
                rstd = stats.tile([128, 2, JB], f32, tag="rstd")
                nc.vector.reciprocal(out=rstd, in_=std)
                bias_t = stats.tile([128, 2, JB], f32, tag="bt")
                nc.vector.scalar_tensor_tensor(
                    out=bias_t,
                    in0=mv[:, :, :, 0],
                    scalar=-1.0,
                    in1=rstd,
                    op0=AOP.mult,
                    op1=AOP.mult,
                )

                for i in range(2):
                    e_row = e_pair[:, i, :, :]
                    # xhat = (e - mean)*rstd ; split across DVE and ScalarE
                    xh = work.tile([128, JB, F], bf16, tag="xh")
                    for j in range(2):
                        nc.vector.tensor_scalar(
                            out=xh[:, j, :],
                            in0=e_row[:, j, :],
                            scalar1=mv[:, i, j, 0:1],
                            scalar2=rstd[:, i, j : j + 1],
                            op0=AOP.subtract,
                            op1=AOP.mult,
                        )
                    for j in range(2, JB):
                        nc.scalar.activation(
                            out=xh[:, j, :],
                            in_=e_row[:, j, :],
                            func=ACTF.Identity,
                            bias=bias_t[:, i, j : j + 1],
                            scale=rstd[:, i, j : j + 1],
                        )
                    # transpose xhat via PE -> [f, (j tok)]
                    xt = work.tile([128, JB, 128], bf16, tag="xt")
                    xtp = psx.tile([128, JB, 128], bf16, tag="xtp")
                    for j in range(JB):
                        nc.tensor.transpose(xtp[:, j, :], xh[:, j, :], ident_bf)
                    nc.scalar.copy(out=xt, in_=xtp)

                    # t1: h_T[dt_k] = W1g_k^T @ xhat_T  (N=512)
                    h_sb = work.tile([128, NT, A], bf16, tag="h")
                    for k in range(NT):
                        ph = psh.tile([128, A], f32, tag="ph")
                        nc.tensor.matmul(
                            ph,
                            lhsT=w1_sb[:, ts(k, 128)],
                            rhs=xt,
                            start=True,
                            stop=True,
                        )
                        nc.scalar.activation(
                            out=h_sb[:, k, :],
                            in_=ph,
                            func=ACTF.Relu,
                            bias=b1_sb[:, k : k + 1],
                            scale=1.0,
                        )
                    # t2 (f-major): poT[f, tok] = sum_k W2_k^T @ h_k
                    poT = pso.tile([128, A], f32, tag="poT")
                    for k in range(NT):
                        nc.tensor.matmul(
                            poT,
                            lhsT=w2_sb[:, k, :],
                            rhs=h_sb[:, k, :],
                            start=(k == 0),
                            stop=(k == NT - 1),
                        )
                    # evacuate as bf16 (+b2 via bias port), transpose on PE
                    po_bf = work.tile([128, JB, 128], bf16, tag="po")
                    nc.scalar.activation(
                        out=po_bf,
                        in_=poT,
                        func=ACTF.Identity,
                        bias=b2_sb[:, 0:1],
                        scale=1.0,
                    )
                    pot = pst.tile([128, JB, F], bf16, tag="pot")
                    for j in range(JB):
                        nc.tensor.transpose(pot[:, j, :], po_bf[:, j, :], ident_bf)
                    ot = io.tile([128, JB, F], f32, tag="ot")
                    nc.vector.tensor_add(out=ot, in0=pot, in1=e_row)
                    nc.sync.dma_start(
                        out=edge_out[a0 + i].rearrange("(j p) f -> p j f", p=128),
                        in_=ot,
                    )

    _split_waits(nc)
    return nc


def _layer_norm_np(x, g, b, eps=EPS):
    m = x.mean(axis=-1, keepdims=True)
    v = ((x - m) ** 2).mean(axis=-1, keepdims=True)
    return (x - m) / np.sqrt(v + eps) * g + b


def _weff(p, mod):
    W = np.asarray(p["W"], np.float32)
    din, dout = W.shape
    Am = (mod @ np.asarray(p["Wa"], np.float32)).reshape(din, R)
    Bm = (mod @ np.asarray(p["Wb"], np.float32)).reshape(R, dout)
    return W + Am @ Bm, np.asarray(p["b"], np.float32)


def kernel(node_vec, edge_vec, node_mask, edge_mask, modulated_params, params):
    global LAST_RESULTS
    node_vec = np.asarray(node_vec, np.float32)
    edge_vec = np.asarray(edge_vec, np.float32)
    node_mask = np.asarray(node_mask, np.float32)
    mod = np.asarray(modulated_params, np.float32)

    # ---- host-side prep (tiny node-level math + weight folding) ----
    ln1_g = np.asarray(params["ln1_g"], np.float32)
    ln1_b = np.asarray(params["ln1_b"], np.float32)
    ln2_g = np.asarray(params["ln2_g"], np.float32)
    ln2_b = np.asarray(params["ln2_b"], np.float32)

    Wl, bl = _weff(params["left"], mod)
    Wr, br = _weff(params["right"], mod)
    Wo, bo = _weff(params["outp"], mod)
    W1, b1 = _weff(params["t1"], mod)
    W2, b2 = _weff(params["t2"], mod)

    act = _layer_norm_np(node_vec, ln1_g, ln1_b)  # [A,F]
    left = (act @ Wl + bl) * node_mask[:, None]  # [A,C]
    right = (act @ Wr + br) * node_mask[:, None]  # [A,C]

    # M[a,d,f] = sum_c left[a,c] * Wo[c*C+d, f]; bias row carries outp bias
    W3 = Wo.reshape(C, C, F)
    Mmat = np.einsum("ac,cdf->adf", left, W3, optimize=True)  # [A,C,F]
    M_aug = np.concatenate(
        [Mmat, np.broadcast_to(bo, (A, 1, F))], axis=1
    )  # [A,C+1,F]
    right_aug = np.concatenate([right, np.ones((A, 1), np.float32)], axis=1)

    # fold ln2 gamma/beta into t1
    W1g = ln2_g[:, None] * W1  # [F,DT]
    b1p = b1 + ln2_b @ W1  # [DT]

    right_t = np.ascontiguousarray(right_aug.T).astype(BF16)  # [C+1, A]
    w1_bf = np.ascontiguousarray(W1g).astype(BF16)  # [F, DT]
    w2_bf = np.ascontiguousarray(
        W2.reshape(NT, F, F).transpose(1, 0, 2)
    ).astype(BF16)  # [F, NT, F] : w2_bf[p,k,f] = W2[k*F+p, f]
    b1_h = np.ascontiguousarray(b1p.reshape(NT, F).T).astype(np.float32)  # [F,NT]
    b2_h = np.ascontiguousarray(b2.reshape(F, 1)).astype(np.float32)

    if "nc" not in _cached:
        _cached["nc"] = _build_bass()
    nc = _cached["nc"]

    in_maps = []
    for i in range(NCORES):
        sl = slice(i * ASH, (i + 1) * ASH)
        m_shard = np.ascontiguousarray(M_aug[sl].transpose(1, 0, 2)).astype(BF16)
        in_maps.append(
            {
                "edge_in": np.ascontiguousarray(edge_vec[sl]),
                "m_t": m_shard,
                "right_t": right_t,
                "w1": w1_bf,
                "w2": w2_bf,
                "b1": b1_h,
                "b2": b2_h,
            }
        )

    trace = os.environ.get("KERNEL_TRACE", "0") == "1"
    kwargs = {}
    if trace:
        kwargs["trace"] = True
        tmpdir = os.environ.get("KERNEL_TRACE_DIR")
        if tmpdir:
            kwargs["tmpdir"] = tmpdir
    res = run_bass_kernel_spmd(nc, in_maps, core_ids=list(range(NCORES)), **kwargs)
    LAST_RESULTS = res

    out = np.concatenate([res.results[i]["edge_out"] for i in range(NCORES)], axis=0)
    return out.astype(np.float32)
